# revision 1
# baseline (speedup 1.0000x reference)
"""Bidirectional masked GRU encoder (Keras reset_after semantics) on 8 trn2 cores.

Sharding: 2 directions x 4 batch-groups (16 batch rows per core, one GRU
direction per core). Each core holds its direction's full recurrent matrix U
(bf16) in SBUF and runs the whole 128-step scan locally - no cross-core
communication.

Tricks:
  - Embedding + input projection fused: EW = emb_table @ W is computed on
    device ([128 vocab, 3072]); per step the input projection is a single
    K=128 one-hot matmul accumulated directly into the recurrence PSUM.
  - Masking is free: EW row 0 (pad token) z-columns are poisoned to +30.0,
    so z = sigmoid(30+rz) == 1.0 exactly in fp32 -> h_new == h on masked
    steps. (For this wiring out_t == h_t identically, masked or not.)
  - Each step is split into two 512-unit halves (wavefront): PSUM ping-pong,
    gates of one half overlap matmuls of the other.
"""

import numpy as np
import ml_dtypes

import concourse.bass as bass
import concourse.mybir as mybir
from concourse import bass_utils

BF16 = ml_dtypes.bfloat16
B, T, UNITS, VOCAB = 64, 128, 1024, 128
BL = 16          # batch rows per core
NK = 8           # K tiles of the 1024-unit contraction
HU = 512         # units per half
dt = mybir.dt
AF = mybir.ActivationFunctionType
OP = mybir.AluOpType

_compiled = {}


def _build_nc(nsteps=T, nodma=False):
    nc = bass.Bass("TRN2")

    d_u = nc.dram_tensor("u_t", [NK, 128, 3 * UNITS], dt.bfloat16, kind="ExternalInput")
    d_w = nc.dram_tensor("w_t", [NK, 128, 3 * UNITS], dt.bfloat16, kind="ExternalInput")
    d_et = nc.dram_tensor("et_t", [NK, 128, VOCAB], dt.bfloat16, kind="ExternalInput")
    d_oh = nc.dram_tensor("oh_t", [VOCAB, T * BL], dt.bfloat16, kind="ExternalInput")
    d_id = nc.dram_tensor("id_t", [128, 128], dt.float32, kind="ExternalInput")
    d_out = nc.dram_tensor("out_t", [T, 2, 64, 128], dt.float32, kind="ExternalOutput")

    from contextlib import ExitStack
    ctx = ExitStack()
    u_sb = ctx.enter_context(nc.sbuf_tensor([128, NK * 3 * UNITS], dt.bfloat16))
    w_sb = ctx.enter_context(nc.sbuf_tensor([128, NK * 3 * UNITS], dt.bfloat16))
    ew_sb = ctx.enter_context(nc.sbuf_tensor([128, 3 * UNITS], dt.bfloat16))
    et_sb = ctx.enter_context(nc.sbuf_tensor([128, NK * VOCAB], dt.bfloat16))
    oh_sb = ctx.enter_context(nc.sbuf_tensor([128, T * BL], dt.bfloat16))
    id_sb = ctx.enter_context(nc.sbuf_tensor([128, 128], dt.float32))
    # per-half double-buffered temps
    zr_sb = [ctx.enter_context(nc.sbuf_tensor(f"zr_sb{i}", [16, 1024], dt.bfloat16)) for i in range(2)]
    t2_sb = [ctx.enter_context(nc.sbuf_tensor(f"t2_sb{i}", [16, 512], dt.bfloat16)) for i in range(2)]
    t3_sb = [ctx.enter_context(nc.sbuf_tensor(f"t3_sb{i}", [16, 512], dt.bfloat16)) for i in range(2)]
    zd_sb = [ctx.enter_context(nc.sbuf_tensor(f"zd_sb{i}", [64, 128], dt.bfloat16)) for i in range(2)]
    t3d_sb = [ctx.enter_context(nc.sbuf_tensor(f"t3d_sb{i}", [64, 128], dt.bfloat16)) for i in range(2)]
    hh_sb = [ctx.enter_context(nc.sbuf_tensor(f"hh_sb{i}", [64, 128], dt.bfloat16)) for i in range(2)]
    zc_sb = [ctx.enter_context(nc.sbuf_tensor(f"zc_sb{i}", [64, 128], dt.bfloat16)) for i in range(2)]
    a_sb = [ctx.enter_context(nc.sbuf_tensor(f"a_sb{i}", [64, 128], dt.float32)) for i in range(2)]
    b2_sb = [ctx.enter_context(nc.sbuf_tensor(f"b2_sb{i}", [64, 128], dt.float32)) for i in range(2)]
    h_sb = [ctx.enter_context(nc.sbuf_tensor(f"h_sb{i}", [64, 128], dt.float32)) for i in range(2)]
    ht_sb = [ctx.enter_context(nc.sbuf_tensor(f"ht_sb{i}", [128, 64], dt.bfloat16)) for i in range(2)]
    hb_sb = [ctx.enter_context(nc.sbuf_tensor(f"hb_sb{i}", [64, 128], dt.bfloat16)) for i in range(2)]
    ps = [ctx.enter_context(nc.psum_tensor(f"ps{i}", [128, 2048], dt.float32)) for i in range(2)]

    sems = {}
    for name in ["s_load", "s_ewmm", "s_ewcp", "s_mm", "s_sig",
                 "s_t3", "s_rs", "s_tanh", "s_h", "s_tp", "s_cp", "s_od", "s_cast", "s_mmzr"]:
        sems[name] = ctx.enter_context(nc.semaphore(name))
    s_load, s_ewmm, s_ewcp = sems["s_load"], sems["s_ewmm"], sems["s_ewcp"]
    s_mm, s_sig, s_t3, s_rs = sems["s_mm"], sems["s_sig"], sems["s_t3"], sems["s_rs"]
    s_tanh, s_h, s_tp, s_cp, s_od = sems["s_tanh"], sems["s_h"], sems["s_tp"], sems["s_cp"], sems["s_od"]
    s_cast, s_mmzr = sems["s_cast"], sems["s_mmzr"]

    N_LOAD = 3 * NK + 2

    # u_sb per k-tile cols: [z(1024) | r(1024) | h(1024)], each gate = [half0 512 | half1 512]
    u5 = u_sb[:, :].rearrange("p (k gate hf u) -> p k gate hf u", k=NK, gate=3, hf=2)
    ew4 = ew_sb[:, :].rearrange("p (gate hf u) -> p gate hf u", gate=3, hf=2)
    def ht_tile(k):   # [128, 16] stationary for global contraction tile k
        return ht_sb[k // 4][:, 16 * (k % 4): 16 * (k % 4) + 16]

    with nc.Block() as block:

        @block.sync
        def _(sync):
            TT = nsteps
            for k in range(NK):
                sync.dma_start(w_sb[:, 3 * UNITS * k: 3 * UNITS * (k + 1)], d_w[k]).then_inc(s_load, 16)
            for k in range(NK):
                sync.dma_start(et_sb[:, VOCAB * k: VOCAB * (k + 1)], d_et[k]).then_inc(s_load, 16)
            for k in range(NK):
                sync.dma_start(u_sb[:, 3 * UNITS * k: 3 * UNITS * (k + 1)], d_u[k]).then_inc(s_load, 16)
            sync.dma_start(oh_sb[:, :], d_oh[:, :]).then_inc(s_load, 16)
            sync.dma_start(id_sb[:, :], d_id[:, :]).then_inc(s_load, 16)
            for t in range(TT):
                if nodma:
                    break
                for hf in range(2):
                    s = 2 * t + hf
                    # reshape DMAs: [16, 512] -> [64, 128] dense (b*4+blk, u')
                    sync.wait_ge(s_t3, s + 1)
                    for blk in range(4):
                        sync.dma_start(zd_sb[hf][16 * blk: 16 * (blk + 1), :],
                                       zr_sb[hf][:, 128 * blk: 128 * (blk + 1)]).then_inc(s_rs, 16)
                    for blk in range(4):
                        sync.dma_start(t3d_sb[hf][16 * blk: 16 * (blk + 1), :],
                                       t3_sb[hf][:, 128 * blk: 128 * (blk + 1)]).then_inc(s_rs, 16)
                    # output + hT via DMA xbar transpose (bf16)
                    sync.wait_ge(s_h, s + 1)
                    if t < TT - 1:
                        sync.wait_ge(s_cast, s + 1)
                        if hf == 0 and t > 0:
                            sync.wait_ge(s_mm, s + 2)
                        sync.dma_start(ht_sb[hf][:, :], hb_sb[hf][:, :], transpose=True).then_inc(s_cp, 16)
                    sync.dma_start(d_out[t % T, hf], h_sb[hf][:, :]).then_inc(s_od, 16)

        @block.tensor
        def _(pe):
            # EW = E @ W, three 1024-col chunks through ps[0]
            pe.wait_ge(s_load, 16 * (2 * NK))
            for c in range(3):
                if c > 0:
                    pe.wait_ge(s_ewcp, c)
                for nn in range(2):
                    for k in range(NK):
                        base = 3 * UNITS * k + 1024 * c + 512 * nn
                        mm = pe.matmul(
                            ps[0][0:128, 512 * nn: 512 * (nn + 1)],
                            et_sb[:, VOCAB * k: VOCAB * (k + 1)],
                            w_sb[:, base: base + 512],
                            start=(k == 0), stop=(k == NK - 1),
                            skip_group_check=True)
                        if nn == 1 and k == NK - 1:
                            mm.then_inc(s_ewmm, 1)
            pe.wait_ge(s_load, 16 * N_LOAD)
            pe.wait_ge(s_ewcp, 4)   # 3 chunks + poison marker
            TT = nsteps
            for t in range(T if False else TT):
                for hf in range(2):
                    s = 2 * t + hf
                    if t > 0 and not nodma:
                        pe.wait_ge(s_cp, 32 * t)
                    # z,r matmuls first so sigmoid can start early
                    oh_t = oh_sb[:, BL * (t % T): BL * ((t % T) + 1)]
                    zr_last = pe.matmul(ps[hf][0:16, 0:512], oh_t, ew4[:, 0, hf, :],
                                        start=True, stop=(t == 0), skip_group_check=True)
                    zr_last = pe.matmul(ps[hf][0:16, 512:1024], oh_t, ew4[:, 1, hf, :],
                                        start=True, stop=(t == 0), skip_group_check=True)
                    if t > 0:
                        for k in range(NK):
                            pe.matmul(ps[hf][0:16, 0:512], ht_tile(k), u5[:, k, 0, hf, :],
                                      start=False, stop=(k == NK - 1), skip_group_check=True)
                            zr_last = pe.matmul(ps[hf][0:16, 512:1024], ht_tile(k), u5[:, k, 1, hf, :],
                                                start=False, stop=(k == NK - 1), skip_group_check=True)
                    zr_last.then_inc(s_mmzr, 1)
                    last = pe.matmul(ps[hf][0:16, 1536:2048], oh_t, ew4[:, 2, hf, :],
                                     start=True, stop=True, skip_group_check=True)
                    if t > 0:
                        for k in range(NK):
                            last = pe.matmul(ps[hf][0:16, 1024:1536], ht_tile(k), u5[:, k, 2, hf, :],
                                             start=(k == 0), stop=(k == NK - 1), skip_group_check=True)
                    last.then_inc(s_mm, 1)

        @block.scalar
        def _(act):
            TT = nsteps
            for t in range(TT):
                for hf in range(2):
                    s = 2 * t + hf
                    act.wait_ge(s_mmzr, s + 1)
                    if s >= 2 and not nodma:
                        act.wait_ge(s_rs, 128 * (s - 1))
                    act.activation(zr_sb[hf][:, :], ps[hf][0:16, 0:1024], AF.Sigmoid).then_inc(s_sig, 1)
                    if not nodma:
                        act.wait_ge(s_rs, 128 * (s + 1))
                    act.activation(hh_sb[hf][:, :], t3d_sb[hf][:, :], AF.Tanh).then_inc(s_tanh, 1)

        @block.vector
        def _(v):
            for hf in range(2):
                v.memset(h_sb[hf][:, :], 0.0)
            for c in range(3):
                v.wait_ge(s_ewmm, c + 1)
                v.tensor_copy(ew_sb[:, 1024 * c: 1024 * (c + 1)], ps[0][0:128, 0:1024]).then_inc(s_ewcp, 1)
            v.memset(ew_sb[0:1, 0:1024], 30.0).then_inc(s_ewcp, 1)
            TT = nsteps
            for t in range(TT):
                for hf in range(2):
                    s = 2 * t + hf
                    v.wait_ge(s_sig, s + 1)
                    v.wait_ge(s_mm, s + 1)
                    if t == 0:
                        v.memset(ps[hf][0:16, 1024:1536], 0.0)
                    v.tensor_tensor(t2_sb[hf][:, :], zr_sb[hf][:, 512:1024],
                                    ps[hf][0:16, 1024:1536], OP.mult)
                    if s >= 2 and not nodma:
                        v.wait_ge(s_rs, 128 * (s - 1))
                    v.tensor_tensor(t3_sb[hf][:, :], t2_sb[hf][:, :],
                                    ps[hf][0:16, 1536:2048], OP.add).then_inc(s_t3, 1)
                    if not nodma:
                        v.wait_ge(s_rs, 128 * s + 64)
                    v.tensor_scalar(zc_sb[hf][:, :], zd_sb[hf][:, :], -1.0, 1.0, OP.mult, OP.add)
                    v.tensor_tensor(a_sb[hf][:, :], zd_sb[hf][:, :], h_sb[hf][:, :], OP.mult)
                    v.wait_ge(s_tanh, s + 1)
                    v.tensor_tensor(b2_sb[hf][:, :], zc_sb[hf][:, :], hh_sb[hf][:, :], OP.mult)
                    if t > 0 and not nodma:
                        v.wait_ge(s_od, 16 * (s - 1))
                    v.tensor_tensor(h_sb[hf][:, :], a_sb[hf][:, :], b2_sb[hf][:, :], OP.add).then_inc(s_h, 1)
                    if t < TT - 1:
                        v.tensor_copy(hb_sb[hf][:, :], h_sb[hf][:, :]).then_inc(s_cast, 1)

    ctx.close()
    return nc


def _prep_core_inputs(tokens, emb_table, W, U, core):
    d = core // 4
    g = core % 4
    tok = tokens[BL * g: BL * (g + 1), :]
    if d == 1:
        tok = tok[:, ::-1]
    oh = np.zeros((VOCAB, T * BL), np.float32)
    tt = np.asarray(tok).astype(np.int64)
    for b in range(BL):
        oh[tt[b], np.arange(T) * BL + b] = 1.0
    return {
        "u_t": np.ascontiguousarray(U.reshape(NK, 128, 3 * UNITS)).astype(BF16),
        "w_t": np.ascontiguousarray(W.reshape(NK, 128, 3 * UNITS)).astype(BF16),
        "et_t": np.ascontiguousarray(emb_table.T.reshape(NK, 128, VOCAB)).astype(BF16),
        "oh_t": oh.astype(BF16),
        "id_t": np.eye(128, dtype=np.float32),
    }


def kernel(tokens, emb_table, Wf, Uf, bf, Wb, Ub, bb, _trace=False):
    tokens = np.asarray(tokens)
    emb_table = np.asarray(emb_table, dtype=np.float32)
    assert np.max(np.abs(np.asarray(bf))) == 0 and np.max(np.abs(np.asarray(bb))) == 0, \
        "nonzero GRU biases not supported by this kernel"

    if "nc" not in _compiled:
        _compiled["nc"] = _build_nc()
    nc = _compiled["nc"]

    in_maps = []
    for core in range(8):
        W, U = (Wf, Uf) if core < 4 else (Wb, Ub)
        in_maps.append(_prep_core_inputs(tokens, emb_table,
                                         np.asarray(W, np.float32), np.asarray(U, np.float32), core))

    res = bass_utils.run_bass_kernel_spmd(nc, in_maps, core_ids=list(range(8)), trace=_trace)
    global _last_res
    _last_res = res

    out = np.zeros((B, T, UNITS), np.float32)
    for core in range(8):
        o = res.results[core]["out_t"]                       # [T, 2, 64, 128]
        # h[b, 512*hf + 128*blk + u'] = o[t, hf, b*4+blk, u']
        part = o.reshape(T, 2, 4, BL, 128).transpose(3, 0, 1, 2, 4).reshape(BL, T, UNITS)
        d, g = core // 4, core % 4
        if d == 1:
            part = part[:, ::-1, :]
        out[BL * g: BL * (g + 1)] += part
    return out

